# revision 1
# baseline (speedup 1.0000x reference)
"""Pairwise cosine-similarity adjacency (exp(-0.5 * cos_sim)) on 8 trn2 cores.

Input : x [4, 4096, 512] fp32
Output: exp(-0.5 * (xn @ xn.T)) per batch -> [4, 4096, 4096] fp32,
        xn = x / max(||x||_row, 1e-8)

Sharding (symmetry-aware): batch b = core // 2. The 4096x4096 adjacency is
symmetric, so in units of 1024x1024 quarter-blocks Q[i][j] (i,j in 0..3) only
a triangle cover is computed on-device; the host mirrors the rest.

  core even (own rows 0..2047 of batch b) computes
      dtop = rows 0..1023    x cols 0..2047    (Q00, Q01)
      dbot = rows 1024..2047 x cols 1024..2047 (Q11)
      outc = [rows 0..1023    x cols 2048..3071 (Q02);
              rows 1024..2047 x cols 3072..4095 (Q13)]
  core odd runs the same SPMD program fed own = rows 2048..4095 and
      cross = rows [1024..2047, 0..1023], producing Q22/Q23/Q33 and
      Q21, Q30.
  Host mirrors Q01.T, Q02.T, Q13.T, Q21.T, Q30.T into the lower copies.

Per-core pipeline:
  phase 1: 32 row tiles [128,512] (16 own + 16 cross): DMA in, ACT
           Square+accum into packed [128,8] group tiles, batched
           reciprocal+Sqrt -> inv, DVE normalize (cast f32r), PE
           transpose (f32r) into xnT tiles [128, 2048].
  phase 2: 320 f32r matmuls (K=128,M=128,N=512) into [128,1024] PSUM;
           ACT Exp(scale=-0.5) -> SBUF; DMA out.
"""
import sys

sys.path.insert(0, '/opt/trn_rl_repo')

import numpy as np

B, N, D = 4, 4096, 512
N_CORES = 8
R = N // 2      # 2048 own rows per core
Q = N // 4      # 1024 quarter-block size
EPS = 1e-8

_compiled = {}


def _build():
    import concourse.mybir as mybir
    import concourse.tile as tile
    from concourse import bacc
    from concourse.masks import make_identity

    fp32 = mybir.dt.float32
    f32r = mybir.dt.float32r

    nc = bacc.Bacc(trn_type="TRN2", target_bir_lowering=False, debug=False,
                   num_devices=N_CORES)
    xown = nc.dram_tensor("xown", [R, D], fp32, kind="ExternalInput")
    xcross = nc.dram_tensor("xcross", [R, D], fp32, kind="ExternalInput")
    dtop = nc.dram_tensor("dtop", [Q, 2 * Q], fp32, kind="ExternalOutput")
    dbot = nc.dram_tensor("dbot", [Q, Q], fp32, kind="ExternalOutput")
    outc = nc.dram_tensor("outc", [2 * Q, Q], fp32, kind="ExternalOutput")

    K_TILES = D // 128   # 4 contraction chunks
    NW = 1024            # psum accumulate width (2 banks)

    with tile.TileContext(nc) as tc:
        with tc.tile_pool(name="consts", bufs=1) as consts, \
             tc.tile_pool(name="xn_store", bufs=1) as xn_store, \
             tc.tile_pool(name="p1", bufs=6) as p1, \
             tc.tile_pool(name="p1psum", bufs=2, space="PSUM") as p1psum, \
             tc.tile_pool(name="p2psum", bufs=3, space="PSUM") as p2psum, \
             tc.tile_pool(name="p2out", bufs=4) as p2out:

            identf = consts.tile([128, 128], fp32)
            make_identity(nc, identf[:])
            ident = consts.tile([128, 128], f32r)
            nc.vector.tensor_copy(ident[:], identf[:])

            # xnT[k][s]: s=0 own rows transposed, s=1 cross rows transposed
            xnT = [[xn_store.tile([128, 2 * Q], f32r, name=f"xnT_{k}_{s}")
                    for s in range(2)] for k in range(K_TILES)]
            # packed norms^2, groups of 8 row tiles
            sqh = [xn_store.tile([128, 8], fp32, name=f"sqh_{g}")
                   for g in range(4)]
            invh = [xn_store.tile([128, 8], fp32, name=f"invh_{g}")
                    for g in range(4)]

            srcs = [xown, xcross]
            xts = {}

            def phase1_load(r):          # r in 0..31; side s = r // 16
                s, g, j = r // 16, r // 8, r % 8
                row0 = (r % 16) * 128
                xt = p1.tile([128, D], fp32, tag="xt", bufs=18, name=f"xt_{r}")
                nc.sync.dma_start(xt[:], srcs[s].ap()[row0:row0 + 128, :])
                xts[r] = xt
                scratch = p1.tile([128, D], fp32, tag="scratch", bufs=2)
                nc.scalar.activation(scratch[:], xt[:],
                                     mybir.ActivationFunctionType.Square,
                                     accum_out=sqh[g][:, j:j + 1])

            def phase1_inv(g):
                nc.vector.tensor_scalar_max(sqh[g][:], sqh[g][:], EPS * EPS)
                nc.vector.reciprocal(invh[g][:], sqh[g][:])
                nc.scalar.activation(invh[g][:], invh[g][:],
                                     mybir.ActivationFunctionType.Sqrt)

            def phase1_tp(r):
                s, g, j = r // 16, r // 8, r % 8
                c0 = (r % 16) * 128
                xt = xts[r]
                xnrm = p1.tile([128, D], f32r, tag="xnrm")
                nc.vector.tensor_scalar_mul(xnrm[:], xt[:], invh[g][:, j:j + 1])
                for k in range(K_TILES):
                    pt = p1psum.tile([128, 128], f32r, tag="tp")
                    nc.tensor.transpose(pt[:], xnrm[:, k * 128:(k + 1) * 128],
                                        ident[:])
                    nc.vector.tensor_copy(xnT[k][s][:, c0:c0 + 128], pt[:])

            def gemm(m, s, col0, dst, drow0, dcol0):
                """One [128, NW] output tile: own row block m, rhs side s,
                rhs cols col0.., DMA'd to dst[drow0.., dcol0..]."""
                acc = p2psum.tile([128, NW], fp32, tag="acc")
                for nn in range(NW // 512):
                    c = col0 + nn * 512
                    for k in range(K_TILES):
                        nc.tensor.matmul(
                            acc[:, nn * 512:(nn + 1) * 512],
                            xnT[k][0][:, m * 128:(m + 1) * 128],
                            xnT[k][s][:, c:c + 512],
                            start=(k == 0), stop=(k == K_TILES - 1))
                ot = p2out.tile([128, NW], fp32, tag="ot")
                nc.scalar.activation(ot[:], acc[:],
                                     mybir.ActivationFunctionType.Exp,
                                     scale=-0.5)
                nc.sync.dma_start(
                    dst.ap()[drow0:drow0 + 128, dcol0:dcol0 + NW], ot[:])

            for g in range(4):
                for r in range(g * 8, g * 8 + 8):
                    phase1_load(r)
                phase1_inv(g)
                for r in range(g * 8, g * 8 + 8):
                    phase1_tp(r)

            for m in range(8):                      # Q00, Q01
                for gcol in range(2):
                    gemm(m, 0, gcol * Q, dtop, m * 128, gcol * Q)
            for m in range(8, 16):                  # Q11
                gemm(m, 0, Q, dbot, (m - 8) * 128, 0)
            for m in range(16):                     # Q02 / Q13 (cross)
                gemm(m, 1, (m // 8) * Q, outc, m * 128, 0)

    nc.compile()
    return nc


def _in_maps(x):
    maps = []
    for c in range(N_CORES):
        b = c // 2
        xb = x[b]
        if c % 2 == 0:
            maps.append({"xown": xb[0:R],
                         "xcross": np.ascontiguousarray(xb[R:N])})
        else:
            maps.append({"xown": np.ascontiguousarray(xb[R:N]),
                         "xcross": np.concatenate([xb[Q:2 * Q], xb[0:Q]])})
    return maps


def _assemble(results, out):
    for c in range(N_CORES):
        b, odd = c // 2, c % 2
        o = out[b]
        r0 = odd * 2 * Q                  # own-row offset: 0 or 2048
        dtop = results[c]["dtop"]
        dbot = results[c]["dbot"]
        outc = results[c]["outc"]
        o[r0:r0 + Q, r0:r0 + 2 * Q] = dtop
        o[r0 + Q:r0 + 2 * Q, r0 + Q:r0 + 2 * Q] = dbot
        o[r0 + Q:r0 + 2 * Q, r0:r0 + Q] = dtop[:, Q:2 * Q].T
        # cross cols: even core -> [2048.., 3072..]; odd -> [1024.., 0..]
        ccol = [2 * Q, 3 * Q] if not odd else [Q, 0]
        for half in range(2):
            blk = outc[half * Q:(half + 1) * Q]
            rr = r0 + half * Q
            cc = ccol[half]
            o[rr:rr + Q, cc:cc + Q] = blk
            o[cc:cc + Q, rr:rr + Q] = blk.T
    return out


def kernel(x: np.ndarray) -> np.ndarray:
    from concourse.bass_utils import run_bass_kernel_spmd

    x = np.asarray(x, dtype=np.float32)
    assert x.shape == (B, N, D)

    if "nc" not in _compiled:
        _compiled["nc"] = _build()
    nc = _compiled["nc"]

    res = run_bass_kernel_spmd(nc, _in_maps(x), list(range(N_CORES)))
    out = np.empty((B, N, N), dtype=np.float32)
    return _assemble([res.results[c] for c in range(N_CORES)], out)



# revision 12
# speedup vs baseline: 1.3146x; 1.3146x over previous
"""Pairwise cosine-similarity adjacency (exp(-0.5 * cos_sim)) on 8 trn2 cores.

Input : x [4, 4096, 512] fp32
Output: exp(-0.5 * (xn @ xn.T)) per batch -> [4, 4096, 4096] fp32,
        xn = x / max(||x||_row, 1e-8)

Sharding (symmetry-aware): batch b = core // 2; 2 cores per batch, each owning
2048 rows. The 4096x4096 adjacency is symmetric, so only a triangle cover is
computed on-device (at 128-row tile granularity inside the diagonal quarter
blocks); the host mirrors the rest and upcasts bf16 -> fp32.

Per-core pipeline:
  phase 1 (row tiles [128,512] bf16): DMA in, DVE fused square+reduce for row
          norms, inv = 8/norm via ACT Ln+Exp (same act table set as phase-2
          Exp -> one table load), DVE normalize+cast to fp8e4 (values scaled
          x8 so e4m3 stays in normal range), PE fp8 transposes into
          xnT [128, 4, 2048] (k-major for DoubleRow).
  phase 2: fp8e4 DoubleRow matmuls (K=256 per mm) into [128,<=1536] PSUM
          groups; ACT Exp(scale=-1/128) -> bf16 SBUF; DMA out.

Core even (own rows 0..2047 of batch b), core odd (own rows 2048..4095,
cross = rows [1024..2047, 0..1023]) run the same SPMD program.
"""
import sys

sys.path.insert(0, '/opt/trn_rl_repo')

import numpy as np
import ml_dtypes

B, N, D = 4, 4096, 512
N_CORES = 8
R = N // 2      # 2048 own rows per core
Q = N // 4      # 1024 quarter-block size
SCALE = 8.0
LOG_SCALE = float(np.log(SCALE))
EXP_SCALE = -0.5 / (SCALE * SCALE)   # -1/128

_compiled = {}


def _build():
    import concourse.mybir as mybir
    import concourse.tile as tile
    from concourse import bacc
    from concourse.masks import make_identity

    fp32 = mybir.dt.float32
    bf16 = mybir.dt.bfloat16
    fp8 = mybir.dt.float8e4
    u16 = mybir.dt.uint16
    AF = mybir.ActivationFunctionType
    ALU = mybir.AluOpType
    DR = mybir.MatmulPerfMode.DoubleRow

    nc = bacc.Bacc(trn_type="TRN2", target_bir_lowering=False, debug=False,
                   num_devices=N_CORES)
    xown = nc.dram_tensor("xown", [R, D], bf16, kind="ExternalInput")
    xcross = nc.dram_tensor("xcross", [R, D], bf16, kind="ExternalInput")
    # dA: own rows 0..1023 x own cols 0..2047 (triangle from col 128m)
    # dB: own rows 0..1023 x cross cols 0..1023
    # dC: own rows 1024..2047 x own cols 1024..2047 (triangle)
    # dD: own rows 1024..2047 x cross cols 1024..2047
    dA = nc.dram_tensor("dA", [Q, 2 * Q], bf16, kind="ExternalOutput")
    dB = nc.dram_tensor("dB", [Q, Q], bf16, kind="ExternalOutput")
    dC = nc.dram_tensor("dC", [Q, Q], bf16, kind="ExternalOutput")
    dD = nc.dram_tensor("dD", [Q, Q], bf16, kind="ExternalOutput")

    GW = 1536            # psum accumulate group width (3 banks)

    with tile.TileContext(nc) as tc:
        with tc.tile_pool(name="consts", bufs=1) as consts, \
             tc.tile_pool(name="store", bufs=1) as store, \
             tc.tile_pool(name="pin", bufs=6) as pin, \
             tc.tile_pool(name="pxq", bufs=4) as pxq, \
             tc.tile_pool(name="ptp", bufs=2, space="PSUM") as ptp, \
             tc.tile_pool(name="pacc", bufs=2, space="PSUM") as pacc, \
             tc.tile_pool(name="pout", bufs=3) as pout:

            identf = consts.tile([128, 128], fp32)
            make_identity(nc, identf[:])
            identb = consts.tile([128, 128], bf16)
            nc.vector.tensor_copy(identb[:], identf[:])
            lnk = consts.tile([128, 1], fp32)
            nc.vector.memset(lnk[:], LOG_SCALE)

            # xnT[s]: [128 (d-part), 4 (k-chunk), 2048 (row)] fp8, s=0 own
            xnT = [store.tile([128, 4, 2 * Q], fp8, name=f"xnT_{s}")
                   for s in range(2)]
            sq = store.tile([128, 32], fp32)     # row norms^2, col = tile idx
            logt = store.tile([128, 32], fp32)
            inv8 = store.tile([128, 32], fp32)   # 8 / norm

            srcs = [xown, xcross]
            xts = {}

            def load(r):
                s, row0 = r // 16, (r % 16) * 128
                xt = pin.tile([128, D], bf16, tag="xt", bufs=10)
                nc.sync.dma_start(xt[:], srcs[s].ap()[row0:row0 + 128, :])
                scr = pin.tile([128, D], bf16, tag="scr", bufs=2)
                nc.vector.scalar_tensor_tensor(
                    out=scr[:], in0=xt[:], scalar=1.0, in1=xt[:],
                    op0=ALU.mult, op1=ALU.mult,
                    accum_out=sq[:, r:r + 1])
                xts[r] = xt

            def inv_group(c0):
                cs = slice(c0, c0 + 8)
                nc.vector.tensor_scalar_max(sq[:, cs], sq[:, cs], 1e-16)
                nc.scalar.activation(logt[:, cs], sq[:, cs], AF.Ln)
                # 8 * rsqrt(s) = exp(-0.5*ln(s) + ln 8); same table set as Exp
                nc.scalar.activation(inv8[:, cs], logt[:, cs], AF.Exp,
                                     bias=lnk[:, 0:1], scale=-0.5)

            def norm_transpose(r):
                s, row0 = r // 16, (r % 16) * 128
                xt = xts.pop(r)
                xq = pxq.tile([128, D], bf16, tag="xq")
                nc.vector.tensor_scalar_mul(xq[:], xt[:], inv8[:, r:r + 1])
                pt = ptp.tile([128, 4, 128], bf16, tag="tp")
                for k in range(4):
                    nc.tensor.transpose(pt[:, k, :], xq[:, k * 128:(k + 1) * 128],
                                        identb[:])
                # PSUM->SBUF copy casts bf16 -> fp8e4
                nc.vector.tensor_copy(xnT[s][:, :, row0:row0 + 128],
                                      pt[:, :, :])

            def group(m, side, sc, w, dst, dr0, dc0):
                """One PSUM accumulation group (own row tile m, one output
                segment of width w <= GW), exp'd in one ACT call."""
                assert w <= GW
                acc = pacc.tile([128, GW], fp32, tag="acc")
                mcol = m * 128
                for kp in range(2):
                    lhs = xnT[0][:, 2 * kp:2 * kp + 2, mcol:mcol + 128]
                    for off in range(0, w, 512):
                        cw = min(512, w - off)
                        nc.tensor.matmul(
                            acc[:, off:off + cw],
                            lhs,
                            xnT[side][:, 2 * kp:2 * kp + 2,
                                      sc + off:sc + off + cw],
                            start=(kp == 0), stop=(kp == 1),
                            perf_mode=DR)
                ot = pout.tile([128, GW], bf16, tag="ot")
                nc.scalar.activation(ot[:, :w], acc[:, :w], AF.Exp,
                                     scale=EXP_SCALE)
                nc.sync.dma_start(dst.ap()[dr0:dr0 + 128, dc0:dc0 + w],
                                  ot[:, :w])

            # ---- emission order = scheduling priority ----
            # own rows 1024..2047 first so dC (own-only) matmuls start early
            for r in range(8, 16):
                load(r)
            inv_group(8)
            for r in range(8, 16):
                norm_transpose(r)

            # dC triangle, m=8..15 (widths 1024 down to 128)
            for m in range(8, 16):
                mm = m - 8
                w = Q - 128 * mm
                group(m, 0, Q + 128 * mm, w, dC, 128 * mm, 128 * mm)

            for r in range(0, 8):
                load(r)
            inv_group(0)
            for r in range(0, 8):
                norm_transpose(r)

            # dA rows m=0..7: cols [128m, 2048); split into <=GW pieces
            for m in range(0, 8):
                w = 2 * Q - 128 * m
                c0 = 128 * m
                for off in range(0, w, GW):
                    pw = min(GW, w - off)
                    group(m, 0, c0 + off, pw, dA, 128 * m, c0 + off)

            # cross side
            for r in range(16, 24):
                load(r)
            inv_group(16)
            for r in range(16, 24):
                norm_transpose(r)

            # dB: own rows m=0..7 x cross cols 0..1023
            for m in range(0, 8):
                group(m, 1, 0, Q, dB, 128 * m, 0)

            for r in range(24, 32):
                load(r)
            inv_group(24)
            for r in range(24, 32):
                norm_transpose(r)

            # dD: own rows m=8..15 x cross cols 1024..2047
            for m in range(8, 16):
                mm = m - 8
                group(m, 1, Q, Q, dD, 128 * mm, 0)

    nc.compile()
    return nc


def _in_maps(x):
    xb = x.astype(ml_dtypes.bfloat16)
    maps = []
    for c in range(N_CORES):
        b = c // 2
        xbb = xb[b]
        if c % 2 == 0:
            maps.append({"xown": xbb[0:R],
                         "xcross": np.ascontiguousarray(xbb[R:N])})
        else:
            maps.append({"xown": np.ascontiguousarray(xbb[R:N]),
                         "xcross": np.concatenate([xbb[Q:2 * Q], xbb[0:Q]])})
    return maps


_M128 = None


def _assemble(results, out):
    global _M128
    if _M128 is None:
        blk = np.arange(Q) // 128
        _M128 = blk[:, None] <= blk[None, :]
    for c in range(N_CORES):
        b, odd = c // 2, c % 2
        o = out[b]
        r0 = odd * 2 * Q
        A = results[c]["dA"].astype(np.float32)
        Bm = results[c]["dB"].astype(np.float32)
        C = results[c]["dC"].astype(np.float32)
        Dm = results[c]["dD"].astype(np.float32)
        U = A[:, 0:Q]
        o[r0:r0 + Q, r0:r0 + Q] = np.where(_M128, U, U.T)
        o[r0:r0 + Q, r0 + Q:r0 + 2 * Q] = A[:, Q:2 * Q]
        o[r0 + Q:r0 + 2 * Q, r0:r0 + Q] = A[:, Q:2 * Q].T
        o[r0 + Q:r0 + 2 * Q, r0 + Q:r0 + 2 * Q] = np.where(_M128, C, C.T)
        bcol = 2 * Q if not odd else Q
        o[r0:r0 + Q, bcol:bcol + Q] = Bm
        o[bcol:bcol + Q, r0:r0 + Q] = Bm.T
        dcol = 3 * Q if not odd else 0
        o[r0 + Q:r0 + 2 * Q, dcol:dcol + Q] = Dm
        o[dcol:dcol + Q, r0 + Q:r0 + 2 * Q] = Dm.T
    return out


def kernel(x: np.ndarray) -> np.ndarray:
    from concourse.bass_utils import run_bass_kernel_spmd

    x = np.asarray(x, dtype=np.float32)
    assert x.shape == (B, N, D)

    if "nc" not in _compiled:
        _compiled["nc"] = _build()
    nc = _compiled["nc"]

    res = run_bass_kernel_spmd(nc, _in_maps(x), list(range(N_CORES)))
    out = np.empty((B, N, N), dtype=np.float32)
    return _assemble([res.results[c] for c in range(N_CORES)], out)
